# revision 28
# baseline (speedup 1.0000x reference)
"""AttentionRNN Trainium2 kernel: 8-core SPMD, vocab-split fc projection.

Self-contained: kernel(**inputs) takes full inputs, returns full [B,S,V] output.
Strategy: every core runs the identical embed+xproj+RNN+attention program
(replicated; the RNN scan is latency-bound so data-parallelism would not help),
and computes a 1/8 vocab slice of the final fc projection (the dominant cost,
537 GFLOP total). No collectives needed; host concatenates the vocab slices.
All matmuls in bf16 with f32 PSUM accumulation (measured end-to-end rel err
~3.5e-3 vs f32 reference).
"""
import sys
if '/opt/trn_rl_repo' not in sys.path:
    sys.path.insert(0, '/opt/trn_rl_repo')

import numpy as np
import ml_dtypes

import concourse.bass as bass
import concourse.mybir as mybir
import concourse.tile as tile
from concourse import bacc
from concourse.bass_utils import run_bass_kernel_spmd
from concourse.masks import make_identity

DT = mybir.dt
BF = DT.bfloat16
F32 = DT.float32
BF_NP = ml_dtypes.bfloat16

VOCAB, H, B, S = 32000, 512, 16, 512
NCORES = 8
VS = VOCAB // NCORES          # 4000 vocab rows per core
TOK = B * S                   # 8192 tokens, order tok = t*16 + b
KH = H // 128                 # 4 h-chunks
KD = (2 * H) // 128           # 8 d-chunks of combined
FC_VW = 512                   # fc vocab chunk width
NVB = (VS + FC_VW - 1) // FC_VW  # fc vocab chunks per core

# debug dump selector: subset of {"uT", "hsT", "ctxT"}
DEBUG_DUMPS = ()
PHASES = 4


def _vb_width(vb):
    return min(512, VS - vb * 512)


def build_nc(phases=PHASES, dumps=DEBUG_DUMPS, repeat=1):
    nc = bacc.Bacc("TRN2", target_bir_lowering=False, debug=False,
                   num_devices=NCORES)

    # uemb[v] = emb[v] @ Wxh^T + (Wxh_b + Whh_b), precomputed on host, so the
    # gather directly yields the RNN input u and xproj disappears on-device.
    uemb = nc.dram_tensor("uemb", [VOCAB, H], BF, kind="ExternalInput").ap()
    idxw = nc.dram_tensor("idxw", [128, TOK // 16], DT.int16, kind="ExternalInput").ap()
    whhT = nc.dram_tensor("whhT", [128, KH * H], BF, kind="ExternalInput").ap()
    maskT = nc.dram_tensor("maskT", [128, 128], F32, kind="ExternalInput").ap()
    fcwT = nc.dram_tensor("fcwT", [128, NVB * KD * FC_VW], BF, kind="ExternalInput").ap()
    fcb = nc.dram_tensor("fcb", [128, VS], F32, kind="ExternalInput").ap()
    if phases >= 4:
        y = nc.dram_tensor("y", [B, S, VS], BF, kind="ExternalOutput").ap()
    dump_aps = {}
    for name in dumps:
        dump_aps[name] = nc.dram_tensor(
            name + "_dump", [128, KH * TOK], BF, kind="ExternalOutput").ap()

    NT = 512                  # tok chunk for gather + xproj
    NCH = TOK // NT           # 16 chunks
    NSC = 32                  # RNN steps per streamed u chunk
    NUC = S // NSC            # u chunks
    VW = FC_VW                # fc vocab chunk width
    NVB2 = NVB

    with tile.TileContext(nc) as tc:
      for _rep in range(repeat):
        with tc.tile_pool(name="perm", bufs=1) as perm:
            hsT = perm.tile([128, KH * TOK], BF, tag="hsT")
            ident = perm.tile([128, 128], BF, tag="ident")
            make_identity(nc, ident[:])

            # [128, KH, TOK] views; free index = t*16+b
            hsT3 = hsT[:].rearrange("p (k n) -> p k n", k=KH)
            hsT4 = hsT[:].rearrange("p (k t b) -> p k t b", k=KH, b=B)
            hsT_t = hsT[:].rearrange("p (k t b) -> p t k b", k=KH, b=B)

            # ------- phases 1+2 merged: per-chunk u gather + RNN scan ------
            # The RNN-persistent pools (whh, ub, ps_r) stay open around
            # phases 3/4 so attention/fc overlap the RNN tail without
            # pool-close WAR serialization.
            with tc.tile_pool(name="p_rnn", bufs=1) as p_rnn, \
                 tc.tile_pool(name="p_ub", bufs=2) as p_ub, \
                 tc.tile_pool(name="ps_r", bufs=1, space="PSUM") as ps_r:
                whh_sb = p_rnn.tile([128, KH * H], BF, tag="whh")
                idx_sb = p_rnn.tile([128, TOK // 16], DT.int16, tag="idx")
                nc.sync.dma_start(out=whh_sb[:], in_=whhT[:])
                nc.sync.dma_start(out=idx_sb[:], in_=idxw[:])

                def emit_rnn_chunk(c):
                    ub = p_ub.tile([128, KH * NSC * B], BF, tag="ub")
                    ub3 = ub[:].rearrange("p (k n) -> p k n", k=KH)
                    # gather layout: [p, k, i] = uemb[tok_i, k*128+p]
                    nc.gpsimd.dma_gather(
                        out_ap=ub3[:, :, :],
                        in_ap=uemb[:],
                        idxs_ap=idx_sb[:, c * (NT // 16):(c + 1) * (NT // 16)],
                        num_idxs=NT,
                        num_idxs_reg=NT,
                        elem_size=H,
                        transpose=True,
                        single_packet=False,
                    )
                    for t in range(c * NSC, (c + 1) * NSC):
                        tl = (t - c * NSC) * B
                        if t == 0:
                            ub_t0 = ub[:].rearrange(
                                "p (k t b) -> p t k b", k=KH, b=B)
                            nc.scalar.activation(
                                hsT_t[:, 0], ub_t0[:, 0],
                                mybir.ActivationFunctionType.Tanh)
                            continue
                        prev = slice((t - 1) * B, t * B)
                        # one psum bank holds all 4 m-chunks [128, 4*16]
                        pm = ps_r.tile([128, KH * B], F32, tag="pr")
                        pm2 = pm[:].rearrange("p (k b) -> p k b", k=KH)
                        for mg in range(KH):
                            nc.tensor.matmul(
                                pm2[:, mg], lhsT=ident[:],
                                rhs=ub3[:, mg, tl:tl + B],
                                start=True, stop=False)
                            for k in range(KH):
                                nc.tensor.matmul(
                                    pm2[:, mg],
                                    lhsT=whh_sb[:, k * H + mg * 128:k * H + mg * 128 + 128],
                                    rhs=hsT3[:, k, prev],
                                    start=False, stop=(k == KH - 1))
                        nc.scalar.activation(
                            hsT_t[:, t], pm2[:],
                            mybir.ActivationFunctionType.Tanh)

                if phases >= 2:
                    for c in range(NUC):
                        emit_rnn_chunk(c)

                if "hsT" in dump_aps:
                    nc.sync.dma_start(out=dump_aps["hsT"][:], in_=hsT[:])

                # ------- phases 3+4: block-streamed attention + fc -------
                # tq-blocks of 128 timesteps; block mq only needs hs for
                # t < (mq+1)*128, so attention + fc for early blocks overlap
                # the tail of the RNN.
                if phases >= 3:
                        TB = 128 * B  # 2048 toks per block
                        with tc.tile_pool(name="ph3", bufs=1) as p3, \
                             tc.tile_pool(name="ctxp", bufs=2) as ctxp, \
                             tc.tile_pool(name="p3w", bufs=2) as p3w, \
                             tc.tile_pool(name="fcw", bufs=2) as pfcw, \
                             tc.tile_pool(name="fco", bufs=3) as pfco, \
                             tc.tile_pool(name="ps_s", bufs=2, space="PSUM") as ps_s, \
                             tc.tile_pool(name="ps_t", bufs=1, space="PSUM") as ps_t, \
                             tc.tile_pool(name="ps_c", bufs=1, space="PSUM") as ps_c, \
                             tc.tile_pool(name="ps_o", bufs=3, space="PSUM") as ps_o:
                            # hs in [tk-part, (chunk, b, h)] layout, per block
                            hs_all = p3.tile([128, KH * B * H], BF, tag="hs_all")
                            hs_all4 = hs_all[:].rearrange(
                                "p (c b h) -> p c b h", c=KH, b=B)
                            mask_sb = p3.tile([128, 128], BF, tag="mask")
                            nc.gpsimd.dma_start(out=mask_sb[:], in_=maskT[:])
                            if phases >= 4:
                                fcb_sb = p3.tile([128, VS], BF, tag="fcb")
                                nc.gpsimd.dma_start(out=fcb_sb[:], in_=fcb[:])
                                fcwT3 = fcwT.rearrange("p (vb x) -> p vb x", vb=NVB)
                            for mq in range(KH):
                                ntk = (mq + 1) * 128
                                ctxb = ctxp.tile([128, KH * TB], BF, tag="ctxb")
                                ctxb3 = ctxb[:].rearrange("p (k n) -> p k n", k=KH)
                                ctxb4 = ctxb[:].rearrange(
                                    "p (k t b) -> p k t b", k=KH, b=B)
                                for b in range(B):
                                    # transpose this block's hs chunk (4 kh)
                                    ptt = ps_t.tile([128, KH * 128], BF, tag="ptt")
                                    ptt3 = ptt[:].rearrange("p (k n) -> p k n", k=KH)
                                    for kh in range(KH):
                                        nc.tensor.transpose(
                                            ptt3[:, kh],
                                            hsT4[:, kh, mq * 128:(mq + 1) * 128, b],
                                            ident[:])
                                    nc.vector.tensor_copy(
                                        hs_all4[:, mq, b, :], ptt[:])
                                    # scores, tk <= ntk only (causal skip)
                                    ps = ps_s.tile([128, S], F32, tag="ps")
                                    for kh in range(KH):
                                        nc.tensor.matmul(
                                            ps[:, 0:ntk],
                                            lhsT=hsT4[:, kh, mq * 128:(mq + 1) * 128, b],
                                            rhs=hsT4[:, kh, 0:ntk, b],
                                            start=(kh == 0), stop=(kh == KH - 1))
                                    # mask diag block in place in psum
                                    nc.vector.tensor_tensor(
                                        out=ps[:, mq * 128:ntk],
                                        in0=ps[:, mq * 128:ntk],
                                        in1=mask_sb[:], op=mybir.AluOpType.add)
                                    st = p3w.tile([128, 4], F32, tag="st")
                                    nmx, zs, zi = st[:, 0:1], st[:, 1:2], st[:, 2:3]
                                    nc.vector.reduce_max(
                                        nmx, ps[:, 0:ntk],
                                        axis=mybir.AxisListType.X, negate=True)
                                    es = p3w.tile([128, S], BF, tag="es")
                                    nc.scalar.activation(
                                        es[:, 0:ntk], ps[:, 0:ntk],
                                        mybir.ActivationFunctionType.Exp,
                                        bias=nmx, accum_out=zs)
                                    nc.vector.reciprocal(zi, zs)
                                    w_sb = p3w.tile([128, S], BF, tag="w_sb")
                                    nc.vector.tensor_scalar_mul(
                                        w_sb[:, 0:ntk], es[:, 0:ntk], zi)
                                    # transpose w chunks -> wT [tk-part, 128 tq]
                                    wT = p3w.tile([128, KH * 128], BF, tag="wT")
                                    wT3 = wT[:].rearrange("p (c n) -> p c n", c=KH)
                                    for ktk in range(mq + 1):
                                        nc.sync.dma_start_transpose(
                                            wT3[:, ktk, :],
                                            w_sb[:, ktk * 128:(ktk + 1) * 128])
                                    # contextT block cols for b: [(kh) h, tq]
                                    pc = ps_c.tile([128, KH * 128], F32, tag="pc")
                                    pc3 = pc[:].rearrange("p (k n) -> p k n", k=KH)
                                    for mh in range(KH):
                                        for ktk in range(mq + 1):
                                            nc.tensor.matmul(
                                                pc3[:, mh],
                                                lhsT=hs_all4[:, ktk, b,
                                                             mh * 128:(mh + 1) * 128],
                                                rhs=wT3[:, ktk, :],
                                                start=(ktk == 0), stop=(ktk == mq))
                                    nc.vector.tensor_copy(
                                        ctxb4[:, :, :, b], pc3[:, :, :])
                                # fc for this block: tiles = (one batch, 128
                                # contiguous timesteps) so the y store's DRAM
                                # AP leads with the 128-partition dim.
                                if phases >= 4:
                                    for vb in range(NVB2):
                                        vw = min(VW, VS - vb * VW)
                                        fw = pfcw.tile([128, KD * VW], BF, tag="fw")
                                        fw3 = fw[:].rearrange(
                                            "p (k v) -> p k v", k=KD)
                                        nc.scalar.dma_start(
                                            out=fw[:], in_=fcwT3[:, vb, :])
                                        for b in range(B):
                                            po = ps_o.tile([128, VW], F32, tag="po")
                                            for k in range(KD):
                                                lhsT = (hsT4[:, k, mq * 128:(mq + 1) * 128, b]
                                                        if k < KH else
                                                        ctxb4[:, k - KH, :, b])
                                                nc.tensor.matmul(
                                                    po[:, 0:vw], lhsT=lhsT,
                                                    rhs=fw3[:, k, 0:vw],
                                                    start=(k == 0), stop=(k == KD - 1))
                                            ob = pfco.tile([128, VW], BF, tag="ob")
                                            nc.vector.tensor_tensor(
                                                out=ob[:, 0:vw], in0=po[:, 0:vw],
                                                in1=fcb_sb[:, vb * VW:vb * VW + vw],
                                                op=mybir.AluOpType.add)
                                            nc.sync.dma_start(
                                                out=y[b, mq * 128:(mq + 1) * 128,
                                                      vb * VW:vb * VW + vw],
                                                in_=ob[:, 0:vw])
    nc.compile()
    return nc


# ---------------------------------------------------------------------------
# host side
# ---------------------------------------------------------------------------

def prep_inputs(x, emb, Wxh_w, Wxh_b, Whh_w, Whh_b, fc_w, fc_b):
    """Build per-core in_maps with device layouts."""
    x = np.asarray(x)
    emb = np.asarray(emb, dtype=np.float32)
    Wxh_w = np.asarray(Wxh_w, dtype=np.float32)
    Wxh_b = np.asarray(Wxh_b, dtype=np.float32)
    Whh_w = np.asarray(Whh_w, dtype=np.float32)
    Whh_b = np.asarray(Whh_b, dtype=np.float32)
    fc_w = np.asarray(fc_w, dtype=np.float32)
    fc_b = np.asarray(fc_b, dtype=np.float32)

    # uemb[v] = emb[v] @ Wxh^T + (Wxh_b + Whh_b): the gather then directly
    # yields the RNN pre-activation u (no on-device xproj).
    bias = (Wxh_b + Whh_b).astype(np.float32)
    uemb = np.ascontiguousarray(
        (emb @ Wxh_w.T + bias).astype(BF_NP))
    # idx wrapped: flat tok order = t*16+b ; slot j -> [j%16, j//16]
    idx_flat = np.ascontiguousarray(x.T).reshape(-1).astype(np.int64)  # [S*B] t-major
    wrapped = idx_flat.reshape(TOK // 16, 16).T.astype(np.int16)  # [16, TOK//16]
    # replicated across the 8 gpsimd Q7 cores: each reads its own 16-partition group
    idxw = np.ascontiguousarray(np.tile(wrapped, (8, 1)))

    def pack_T(w):  # w [G, H] -> lhsT layout [128, KH*G] : [p, k*G+g] = w[g, k*128+p]
        wT = np.ascontiguousarray(w.T)            # [H, G]
        kh = wT.shape[0] // 128
        return np.ascontiguousarray(
            wT.reshape(kh, 128, wT.shape[1]).transpose(1, 0, 2).reshape(128, -1)
        ).astype(BF_NP)

    whhT = pack_T(Whh_w)

    p = np.arange(128)[:, None]
    j = np.arange(128)[None, :]
    maskT = np.where(j <= p, 0.0, -1e30).astype(np.float32)

    base = {
        "uemb": uemb, "idxw": idxw, "whhT": whhT, "maskT": maskT,
    }
    in_maps = []
    for c in range(NCORES):
        sl = slice(c * VS, (c + 1) * VS)
        fcwT_kv = pack_T(fc_w[sl]).reshape(128, KD, VS)   # [p, k, v]
        # vb-major contiguous: [p, vb, k, FC_VW] (zero-padded last chunk)
        fcwT = np.zeros((128, NVB, KD, FC_VW), BF_NP)
        for vb in range(NVB):
            vw = min(FC_VW, VS - vb * FC_VW)
            fcwT[:, vb, :, :vw] = fcwT_kv[:, :, vb * FC_VW:vb * FC_VW + vw]
        fcwT = np.ascontiguousarray(fcwT.reshape(128, NVB * KD * FC_VW))
        fcb_bc = np.ascontiguousarray(
            np.broadcast_to(fc_b[sl].astype(np.float32), (128, VS)))
        m = dict(base)
        m["fcwT"] = fcwT
        m["fcb"] = fcb_bc
        in_maps.append(m)
    return in_maps


_NC_CACHE = {}


def get_nc(phases=PHASES, dumps=DEBUG_DUMPS):
    key = (phases, tuple(dumps))
    if key not in _NC_CACHE:
        _NC_CACHE[key] = build_nc(phases, dumps)
    return _NC_CACHE[key]


def kernel(x, emb, Wxh_w, Wxh_b, Whh_w, Whh_b, fc_w, fc_b):
    nc = get_nc()
    in_maps = prep_inputs(x, emb, Wxh_w, Wxh_b, Whh_w, Whh_b, fc_w, fc_b)
    res = run_bass_kernel_spmd(nc, in_maps, list(range(NCORES)))
    y = np.concatenate([res.results[c]["y"] for c in range(NCORES)], axis=2)
    return np.ascontiguousarray(y.astype(np.float32))



# revision 37
# speedup vs baseline: 1.9497x; 1.9497x over previous
"""AttentionRNN Trainium2 kernel: 8-core SPMD, vocab-split fc projection.

Self-contained: kernel(**inputs) takes full inputs, returns full [B,S,V] output.

Strategy per core (identical program; fc vocab-split 8 ways, no collectives):
- Host precomputes uemb = emb @ Wxh^T + (Wxh_b + Whh_b), so the device token
  gather directly yields the RNN pre-activation u (no on-device xproj).
- RNN scan: u chunks streamed by gpsimd gather; 512 serial steps of
  4x(1+4) [128x128x16] matmuls + one tanh on the Act engine.
- Causal attention in 4 blocks of 128 timesteps; per-batch score/softmax/ctx
  chains software-pipelined; w transposed via XBAR dma-transpose.
- fc projection (the dominant cost, 852us/core at the bf16 roofline) runs in
  (batch, 128-contiguous-timestep) tiles so the y store's DRAM access pattern
  leads with the 128-partition dim (one 2KB-contiguous descriptor per row);
  fc of block mq overlaps the RNN tail + attention of later blocks.
- y stored in bf16 (halves HBM write traffic); host concatenates vocab slices
  and casts to f32. End-to-end rel err ~3.2e-3 vs f32 reference.
"""
import sys
if '/opt/trn_rl_repo' not in sys.path:
    sys.path.insert(0, '/opt/trn_rl_repo')

import numpy as np
import ml_dtypes

import concourse.bass as bass
import concourse.mybir as mybir
import concourse.tile as tile
from concourse import bacc
from concourse.bass_utils import run_bass_kernel_spmd
from concourse.masks import make_identity

DT = mybir.dt
BF = DT.bfloat16
F32 = DT.float32
BF_NP = ml_dtypes.bfloat16

VOCAB, H, B, S = 32000, 512, 16, 512
NCORES = 8
VS = VOCAB // NCORES          # 4000 vocab rows per core
TOK = B * S                   # 8192 tokens, order tok = t*16 + b
KH = H // 128                 # 4 h-chunks
KD = (2 * H) // 128           # 8 d-chunks of combined
FC_VW = 512                   # fc vocab chunk width
NVB = (VS + FC_VW - 1) // FC_VW  # fc vocab chunks per core

# debug dump selector: subset of {"uT", "hsT", "ctxT"}
DEBUG_DUMPS = ()
PHASES = 4


def _vb_width(vb):
    return min(512, VS - vb * 512)


def build_nc(phases=PHASES, dumps=DEBUG_DUMPS, repeat=1):
    nc = bacc.Bacc("TRN2", target_bir_lowering=False, debug=False,
                   num_devices=NCORES)

    # uemb[v] = emb[v] @ Wxh^T + (Wxh_b + Whh_b), precomputed on host, so the
    # gather directly yields the RNN input u and xproj disappears on-device.
    uemb = nc.dram_tensor("uemb", [VOCAB, H], BF, kind="ExternalInput").ap()
    idxw = nc.dram_tensor("idxw", [128, TOK // 16], DT.int16, kind="ExternalInput").ap()
    whhT = nc.dram_tensor("whhT", [128, KH * H], BF, kind="ExternalInput").ap()
    maskT = nc.dram_tensor("maskT", [128, 128], F32, kind="ExternalInput").ap()
    fcwT = nc.dram_tensor("fcwT", [128, NVB * KD * FC_VW], BF, kind="ExternalInput").ap()
    fcb = nc.dram_tensor("fcb", [128, VS], F32, kind="ExternalInput").ap()
    if phases >= 4:
        y = nc.dram_tensor("y", [B, S, VS], BF, kind="ExternalOutput").ap()
    dump_aps = {}
    for name in dumps:
        dump_aps[name] = nc.dram_tensor(
            name + "_dump", [128, KH * TOK], BF, kind="ExternalOutput").ap()

    NT = 512                  # tok chunk for gather + xproj
    NCH = TOK // NT           # 16 chunks
    NSC = 16                  # RNN steps per streamed u chunk
    NUC = S // NSC            # u chunks
    VW = FC_VW                # fc vocab chunk width
    NVB2 = NVB

    with tile.TileContext(nc) as tc:
      for _rep in range(repeat):
        with tc.tile_pool(name="perm", bufs=1) as perm:
            hsT = perm.tile([128, KH * TOK], BF, tag="hsT")
            ident = perm.tile([128, 128], BF, tag="ident")
            make_identity(nc, ident[:])

            # [128, KH, TOK] views; free index = t*16+b
            hsT3 = hsT[:].rearrange("p (k n) -> p k n", k=KH)
            hsT4 = hsT[:].rearrange("p (k t b) -> p k t b", k=KH, b=B)
            hsT_t = hsT[:].rearrange("p (k t b) -> p t k b", k=KH, b=B)

            # ------- phases 1+2 merged: per-chunk u gather + RNN scan ------
            # The RNN-persistent pools (whh, ub, ps_r) stay open around
            # phases 3/4 so attention/fc overlap the RNN tail without
            # pool-close WAR serialization.
            with tc.tile_pool(name="p_rnn", bufs=1) as p_rnn, \
                 tc.tile_pool(name="p_ub", bufs=2) as p_ub, \
                 tc.tile_pool(name="ps_r", bufs=1, space="PSUM") as ps_r:
                whh_sb = p_rnn.tile([128, KH * H], BF, tag="whh")
                idx_sb = p_rnn.tile([128, TOK // 16], DT.int16, tag="idx")
                nc.sync.dma_start(out=whh_sb[:], in_=whhT[:])
                nc.sync.dma_start(out=idx_sb[:], in_=idxw[:])

                def emit_rnn_chunk(c):
                    ub = p_ub.tile([128, KH * NSC * B], BF, tag="ub")
                    ub3 = ub[:].rearrange("p (k n) -> p k n", k=KH)
                    # gather layout: [p, k, i] = uemb[tok_i, k*128+p]
                    nc.gpsimd.dma_gather(
                        out_ap=ub3[:, :, :],
                        in_ap=uemb[:],
                        idxs_ap=idx_sb[:, c * (NSC * B // 16):(c + 1) * (NSC * B // 16)],
                        num_idxs=NSC * B,
                        num_idxs_reg=NSC * B,
                        elem_size=H,
                        transpose=True,
                        single_packet=False,
                    )
                    for t in range(c * NSC, (c + 1) * NSC):
                        tl = (t - c * NSC) * B
                        if t == 0:
                            ub_t0 = ub[:].rearrange(
                                "p (k t b) -> p t k b", k=KH, b=B)
                            nc.scalar.activation(
                                hsT_t[:, 0], ub_t0[:, 0],
                                mybir.ActivationFunctionType.Tanh)
                            continue
                        prev = slice((t - 1) * B, t * B)
                        # one psum bank holds all 4 m-chunks [128, 4*16]
                        pm = ps_r.tile([128, KH * B], F32, tag="pr")
                        pm2 = pm[:].rearrange("p (k b) -> p k b", k=KH)
                        for mg in range(KH):
                            nc.tensor.matmul(
                                pm2[:, mg], lhsT=ident[:],
                                rhs=ub3[:, mg, tl:tl + B],
                                start=True, stop=False)
                            for k in range(KH):
                                nc.tensor.matmul(
                                    pm2[:, mg],
                                    lhsT=whh_sb[:, k * H + mg * 128:k * H + mg * 128 + 128],
                                    rhs=hsT3[:, k, prev],
                                    start=False, stop=(k == KH - 1))
                        nc.scalar.activation(
                            hsT_t[:, t], pm2[:],
                            mybir.ActivationFunctionType.Tanh)

                if phases >= 2:
                    for c in range(NUC):
                        emit_rnn_chunk(c)

                if "hsT" in dump_aps:
                    nc.sync.dma_start(out=dump_aps["hsT"][:], in_=hsT[:])

                # ------- phases 3+4: block-streamed attention + fc -------
                # tq-blocks of 128 timesteps; block mq only needs hs for
                # t < (mq+1)*128, so attention + fc for early blocks overlap
                # the tail of the RNN.
                if phases >= 3:
                        TB = 128 * B  # 2048 toks per block
                        with tc.tile_pool(name="ph3", bufs=1) as p3, \
                             tc.tile_pool(name="ctxp", bufs=2) as ctxp, \
                             tc.tile_pool(name="p3w", bufs=2) as p3w, \
                             tc.tile_pool(name="fcw", bufs=2) as pfcw, \
                             tc.tile_pool(name="fco", bufs=3) as pfco, \
                             tc.tile_pool(name="ps_s", bufs=2, space="PSUM") as ps_s, \
                             tc.tile_pool(name="ps_t", bufs=1, space="PSUM") as ps_t, \
                             tc.tile_pool(name="ps_c", bufs=2, space="PSUM") as ps_c, \
                             tc.tile_pool(name="ps_o", bufs=2, space="PSUM") as ps_o:
                            # hs in [tk-part, (chunk, b, h)] layout, per block
                            hs_all = p3.tile([128, KH * B * H], BF, tag="hs_all")
                            hs_all4 = hs_all[:].rearrange(
                                "p (c b h) -> p c b h", c=KH, b=B)
                            mask_sb = p3.tile([128, 128], BF, tag="mask")
                            nc.gpsimd.dma_start(out=mask_sb[:], in_=maskT[:])
                            if phases >= 4:
                                fcb_sb = p3.tile([128, VS], BF, tag="fcb")
                                nc.gpsimd.dma_start(out=fcb_sb[:], in_=fcb[:])
                                fcwT3 = fcwT.rearrange("p (vb x) -> p vb x", vb=NVB)
                            for mq in range(KH):
                                ntk = (mq + 1) * 128
                                ctxb = ctxp.tile([128, KH * TB], BF, tag="ctxb")
                                ctxb3 = ctxb[:].rearrange("p (k n) -> p k n", k=KH)
                                ctxb4 = ctxb[:].rearrange(
                                    "p (k t b) -> p k t b", k=KH, b=B)
                                # software-pipelined per-batch attention: the
                                # score work of b+1 is emitted between b's
                                # softmax and b's ctx matmuls, so the PE never
                                # idles on a single batch's softmax chain.
                                wT_views = {}

                                def score_part(b):
                                    # transpose this block's hs chunk (4 kh)
                                    ptt = ps_t.tile([128, KH * 128], BF, tag="ptt")
                                    ptt3 = ptt[:].rearrange("p (k n) -> p k n", k=KH)
                                    for kh in range(KH):
                                        nc.tensor.transpose(
                                            ptt3[:, kh],
                                            hsT4[:, kh, mq * 128:(mq + 1) * 128, b],
                                            ident[:])
                                    nc.vector.tensor_copy(
                                        hs_all4[:, mq, b, :], ptt[:])
                                    # scores, tk <= ntk only (causal skip)
                                    ps = ps_s.tile([128, S], F32, tag="ps")
                                    for kh in range(KH):
                                        nc.tensor.matmul(
                                            ps[:, 0:ntk],
                                            lhsT=hsT4[:, kh, mq * 128:(mq + 1) * 128, b],
                                            rhs=hsT4[:, kh, 0:ntk, b],
                                            start=(kh == 0), stop=(kh == KH - 1))
                                    # mask diag block in place in psum
                                    nc.vector.tensor_tensor(
                                        out=ps[:, mq * 128:ntk],
                                        in0=ps[:, mq * 128:ntk],
                                        in1=mask_sb[:], op=mybir.AluOpType.add)
                                    st = p3w.tile([128, 4], F32, tag="st")
                                    nmx, zs, zi = st[:, 0:1], st[:, 1:2], st[:, 2:3]
                                    nc.vector.reduce_max(
                                        nmx, ps[:, 0:ntk],
                                        axis=mybir.AxisListType.X, negate=True)
                                    es = p3w.tile([128, S], BF, tag="es")
                                    nc.scalar.activation(
                                        es[:, 0:ntk], ps[:, 0:ntk],
                                        mybir.ActivationFunctionType.Exp,
                                        bias=nmx, accum_out=zs)
                                    nc.vector.reciprocal(zi, zs)
                                    w_sb = p3w.tile([128, S], BF, tag="w_sb")
                                    nc.vector.tensor_scalar_mul(
                                        w_sb[:, 0:ntk], es[:, 0:ntk], zi)
                                    # transpose w chunks -> wT [tk-part, 128 tq]
                                    wT = p3w.tile([128, KH * 128], BF, tag="wT")
                                    wT3 = wT[:].rearrange("p (c n) -> p c n", c=KH)
                                    for ktk in range(mq + 1):
                                        nc.sync.dma_start_transpose(
                                            wT3[:, ktk, :],
                                            w_sb[:, ktk * 128:(ktk + 1) * 128])
                                    wT_views[b] = wT3

                                def ctx_part(b):
                                    wT3 = wT_views.pop(b)
                                    # contextT block cols for b: [(kh) h, tq]
                                    pc = ps_c.tile([128, KH * 128], F32, tag="pc")
                                    pc3 = pc[:].rearrange("p (k n) -> p k n", k=KH)
                                    for mh in range(KH):
                                        for ktk in range(mq + 1):
                                            nc.tensor.matmul(
                                                pc3[:, mh],
                                                lhsT=hs_all4[:, ktk, b,
                                                             mh * 128:(mh + 1) * 128],
                                                rhs=wT3[:, ktk, :],
                                                start=(ktk == 0), stop=(ktk == mq))
                                    nc.vector.tensor_copy(
                                        ctxb4[:, :, :, b], pc3[:, :, :])

                                for b in range(B):
                                    score_part(b)
                                    if b > 0:
                                        ctx_part(b - 1)
                                ctx_part(B - 1)
                                # fc for this block: tiles = (one batch, 128
                                # contiguous timesteps) so the y store's DRAM
                                # AP leads with the 128-partition dim.
                                if phases >= 4:
                                    for vb in range(NVB2):
                                        vw = min(VW, VS - vb * VW)
                                        fw = pfcw.tile([128, KD * VW], BF, tag="fw")
                                        fw3 = fw[:].rearrange(
                                            "p (k v) -> p k v", k=KD)
                                        nc.scalar.dma_start(
                                            out=fw[:], in_=fcwT3[:, vb, :])
                                        for b in range(B):
                                            po = ps_o.tile([128, VW], F32, tag="po")
                                            for k in range(KD):
                                                lhsT = (hsT4[:, k, mq * 128:(mq + 1) * 128, b]
                                                        if k < KH else
                                                        ctxb4[:, k - KH, :, b])
                                                nc.tensor.matmul(
                                                    po[:, 0:vw], lhsT=lhsT,
                                                    rhs=fw3[:, k, 0:vw],
                                                    start=(k == 0), stop=(k == KD - 1))
                                            ob = pfco.tile([128, VW], BF, tag="ob")
                                            nc.vector.tensor_tensor(
                                                out=ob[:, 0:vw], in0=po[:, 0:vw],
                                                in1=fcb_sb[:, vb * VW:vb * VW + vw],
                                                op=mybir.AluOpType.add)
                                            nc.sync.dma_start(
                                                out=y[b, mq * 128:(mq + 1) * 128,
                                                      vb * VW:vb * VW + vw],
                                                in_=ob[:, 0:vw])
    nc.compile()
    return nc


# ---------------------------------------------------------------------------
# host side
# ---------------------------------------------------------------------------

def prep_inputs(x, emb, Wxh_w, Wxh_b, Whh_w, Whh_b, fc_w, fc_b):
    """Build per-core in_maps with device layouts."""
    x = np.asarray(x)
    emb = np.asarray(emb, dtype=np.float32)
    Wxh_w = np.asarray(Wxh_w, dtype=np.float32)
    Wxh_b = np.asarray(Wxh_b, dtype=np.float32)
    Whh_w = np.asarray(Whh_w, dtype=np.float32)
    Whh_b = np.asarray(Whh_b, dtype=np.float32)
    fc_w = np.asarray(fc_w, dtype=np.float32)
    fc_b = np.asarray(fc_b, dtype=np.float32)

    # uemb[v] = emb[v] @ Wxh^T + (Wxh_b + Whh_b): the gather then directly
    # yields the RNN pre-activation u (no on-device xproj).
    bias = (Wxh_b + Whh_b).astype(np.float32)
    uemb = np.ascontiguousarray(
        (emb @ Wxh_w.T + bias).astype(BF_NP))
    # idx wrapped: flat tok order = t*16+b ; slot j -> [j%16, j//16]
    idx_flat = np.ascontiguousarray(x.T).reshape(-1).astype(np.int64)  # [S*B] t-major
    wrapped = idx_flat.reshape(TOK // 16, 16).T.astype(np.int16)  # [16, TOK//16]
    # replicated across the 8 gpsimd Q7 cores: each reads its own 16-partition group
    idxw = np.ascontiguousarray(np.tile(wrapped, (8, 1)))

    def pack_T(w):  # w [G, H] -> lhsT layout [128, KH*G] : [p, k*G+g] = w[g, k*128+p]
        wT = np.ascontiguousarray(w.T)            # [H, G]
        kh = wT.shape[0] // 128
        return np.ascontiguousarray(
            wT.reshape(kh, 128, wT.shape[1]).transpose(1, 0, 2).reshape(128, -1)
        ).astype(BF_NP)

    whhT = pack_T(Whh_w)

    p = np.arange(128)[:, None]
    j = np.arange(128)[None, :]
    maskT = np.where(j <= p, 0.0, -1e30).astype(np.float32)

    base = {
        "uemb": uemb, "idxw": idxw, "whhT": whhT, "maskT": maskT,
    }
    in_maps = []
    for c in range(NCORES):
        sl = slice(c * VS, (c + 1) * VS)
        fcwT_kv = pack_T(fc_w[sl]).reshape(128, KD, VS)   # [p, k, v]
        # vb-major contiguous: [p, vb, k, FC_VW] (zero-padded last chunk)
        fcwT = np.zeros((128, NVB, KD, FC_VW), BF_NP)
        for vb in range(NVB):
            vw = min(FC_VW, VS - vb * FC_VW)
            fcwT[:, vb, :, :vw] = fcwT_kv[:, :, vb * FC_VW:vb * FC_VW + vw]
        fcwT = np.ascontiguousarray(fcwT.reshape(128, NVB * KD * FC_VW))
        fcb_bc = np.ascontiguousarray(
            np.broadcast_to(fc_b[sl].astype(np.float32), (128, VS)))
        m = dict(base)
        m["fcwT"] = fcwT
        m["fcb"] = fcb_bc
        in_maps.append(m)
    return in_maps


_NC_CACHE = {}


def get_nc(phases=PHASES, dumps=DEBUG_DUMPS):
    key = (phases, tuple(dumps))
    if key not in _NC_CACHE:
        _NC_CACHE[key] = build_nc(phases, dumps)
    return _NC_CACHE[key]


def kernel(x, emb, Wxh_w, Wxh_b, Whh_w, Whh_b, fc_w, fc_b):
    nc = get_nc()
    in_maps = prep_inputs(x, emb, Wxh_w, Wxh_b, Whh_w, Whh_b, fc_w, fc_b)
    res = run_bass_kernel_spmd(nc, in_maps, list(range(NCORES)))
    y = np.concatenate([res.results[c]["y"] for c in range(NCORES)], axis=2)
    return np.ascontiguousarray(y.astype(np.float32))

